# revision 3
# baseline (speedup 1.0000x reference)
"""Causal self-attention (B=1, T=4096, D=1024, H=16, rotate-half RoPE)
as a head-parallel (tensor-parallel) Bass kernel on 8 TRN2 NeuronCores.

Sharding: core c computes heads {2c, 2c+1}:
  - w_qkv column-shard: each core gets its 128 q / 128 k / 128 v columns
  - w_proj row-shard: each core gets rows [128c, 128c+128)
  - each core emits a partial [T, D] output; host sums the 8 partials
    (the "all-reduce after proj" done on the host during unshard).

Per-core layout strategy (all fp32 storage, float32r matmuls):
  - x is passed transposed (xT [D, T]) so every matmul consumes it
    directly (contraction on partitions).
  - q, k are produced transposed+RoPEd: qT2/kT2 [128=2*64hd, T].
  - v is produced natural: v2s [128=T-tile, n_kt, 65*2] with a ones
    column appended per head (gives the softmax denominator for free).
  - scores are computed transposed: S^T [k, q] = kT.T @ qT, exp on ACT
    (scale=1/8 folded in), causal masking via multiplicative 0/1 masks
    on diagonal tiles only (off-causal tiles are never computed).
  - O^T [hd, q] = (v|1).T @ exp(S^T), accumulated over k tiles in PSUM;
    row 64 is the denominator; normalize with DVE reciprocal +
    gpsimd partition-broadcast.
  - proj: out[t, :] = yT2[:, t-tile].T @ w_proj_shard, evacuated
    PSUM->SBUF->HBM.
"""

import math

import numpy as np

B, T, D = 1, 4096, 1024
H = 16
HD = D // H  # 64
N_CORES = 8
CH = 512  # q-chunk (free dim of S^T matmuls)
KT = 128  # k-tile (contraction tile of the PV matmul)

_CACHE: dict = {}


def _ensure_ntff_hook():
    """Register antenv.axon_hooks if the image lacks it (needed only for
    trace=True profiling; harmless otherwise)."""
    import sys
    import types

    try:
        from antenv.axon_hooks import get_axon_ntff_profile_hook  # noqa: F401

        return
    except ImportError:
        pass
    mod = types.ModuleType("antenv.axon_hooks")
    _state = {"hook": None}
    mod.set_axon_ntff_profile_hook = lambda h: _state.__setitem__("hook", h)
    mod.get_axon_ntff_profile_hook = lambda: _state["hook"]
    sys.modules["antenv.axon_hooks"] = mod
    try:
        import antenv

        antenv.axon_hooks = mod
    except ImportError:
        pass
    try:
        from trn_agent_boot.trn_boot import _ntff_profile_via_ctypes

        hook = _ntff_profile_via_ctypes("/opt/axon/libaxon_pjrt.so")
        mod.set_axon_ntff_profile_hook(hook)
    except Exception:
        pass


def build(t=T):
    """Build + compile the per-core SPMD Bass program. Identical program on
    every core; only the weight shards in the input map differ."""
    import concourse.bacc as bacc
    import concourse.mybir as mybir
    import concourse.tile as tile

    f32 = mybir.dt.float32
    f32r = mybir.dt.float32r
    MUL = mybir.AluOpType.mult
    ADD = mybir.AluOpType.add
    EXP = mybir.ActivationFunctionType.Exp

    n_ch = t // CH  # q-chunks
    n_kt = t // KT  # k-tiles / T-tiles
    n_din = D // 128  # contraction tiles over model dim
    kpc = CH // KT  # k-tiles per q-chunk (4)

    nc = bacc.Bacc(
        "TRN2", target_bir_lowering=False, debug=False, num_devices=N_CORES
    )
    xT = nc.dram_tensor("xT", [D, t], f32r, kind="ExternalInput").ap()
    wq2 = nc.dram_tensor("wq2", [D, 128], f32r, kind="ExternalInput").ap()
    wk2 = nc.dram_tensor("wk2", [D, 128], f32r, kind="ExternalInput").ap()
    wv2 = nc.dram_tensor("wv2", [D, 128], f32r, kind="ExternalInput").ap()
    wp2 = nc.dram_tensor("wp2", [128, D], f32r, kind="ExternalInput").ap()
    cos2 = nc.dram_tensor("cos2", [128, t], f32, kind="ExternalInput").ap()
    sin2 = nc.dram_tensor("sin2", [128, t], f32, kind="ExternalInput").ap()
    mask = nc.dram_tensor("mask", [128, kpc, CH], f32, kind="ExternalInput").ap()
    out = nc.dram_tensor("out", [t, D], f32, kind="ExternalOutput").ap()

    with tile.TileContext(nc) as tc:
        with (
            tc.tile_pool(name="w", bufs=1) as pw,
            tc.tile_pool(name="big", bufs=1) as pb,
        ):
            wq2s = pw.tile([128, n_din, 128], f32r, tag="wq")
            nc.sync.dma_start(wq2s[:], wq2.rearrange("(a p) m -> p a m", p=128))
            wk2s = pw.tile([128, n_din, 128], f32r, tag="wk")
            nc.sync.dma_start(wk2s[:], wk2.rearrange("(a p) m -> p a m", p=128))
            wv2s = pw.tile([128, n_din, 128], f32r, tag="wv")
            nc.sync.dma_start(wv2s[:], wv2.rearrange("(a p) m -> p a m", p=128))
            wp2s = pw.tile([128, D], f32r, tag="wp")
            nc.sync.dma_start(wp2s[:], wp2)
            mask4s = pw.tile([128, kpc, CH], f32, tag="mask")
            nc.sync.dma_start(mask4s[:], mask)

            qT2 = pb.tile([128, t], f32r, tag="qT2")
            kT2 = pb.tile([128, t], f32r, tag="kT2")
            v2s = pb.tile([128, n_kt, 130], f32r, tag="v2s")
            yT2 = pb.tile([128, t], f32r, tag="yT2")
            ones = pw.tile([128, n_kt, 1], f32, tag="ones")
            nc.vector.memset(ones[:], 1.0)
            nc.vector.tensor_copy(v2s[:, :, 64:65], ones[:])
            nc.vector.tensor_copy(v2s[:, :, 129:130], ones[:])

            # ---- phase 1: qkv projection + RoPE ----
            with (
                tc.tile_pool(name="rope", bufs=1) as pr,
                tc.tile_pool(name="x", bufs=2) as px,
                tc.tile_pool(name="rt", bufs=2) as prt,
                tc.tile_pool(name="qkps", bufs=3, space="PSUM") as pqk,
                tc.tile_pool(name="vps", bufs=2, space="PSUM") as pv,
            ):
                coss = pr.tile([128, t], f32, tag="cos")
                nc.sync.dma_start(coss[:], cos2)
                s2s = pr.tile([128, t], f32, tag="sin")
                nc.sync.dma_start(s2s[:], sin2)
                xTr = xT.rearrange("(a p) n -> p a n", p=128)
                for ci in range(n_ch):
                    csl = slice(ci * CH, (ci + 1) * CH)
                    xts = px.tile([128, n_din, CH], f32r, tag="x")
                    nc.sync.dma_start(xts[:], xTr[:, :, csl])
                    for dst, wts in ((qT2, wq2s), (kT2, wk2s)):
                        ps_ = pqk.tile([128, CH], f32, tag="qk")
                        for a in range(n_din):
                            nc.tensor.matmul(
                                ps_[:],
                                wts[:, a, :],
                                xts[:, a, :],
                                start=(a == 0),
                                stop=(a == n_din - 1),
                            )
                        # RoPE: dst = ps*cos + shift32(ps)*sin' (sin' sign-folded)
                        nc.vector.tensor_tensor(dst[:, csl], ps_[:], coss[:, csl], MUL)
                        tmp = prt.tile([128, CH], f32, tag="rt")
                        for d0, s0 in ((0, 32), (32, 0), (64, 96), (96, 64)):
                            nc.vector.tensor_tensor(
                                tmp[d0 : d0 + 32, :],
                                ps_[s0 : s0 + 32, :],
                                s2s[d0 : d0 + 32, csl],
                                MUL,
                            )
                        nc.vector.tensor_tensor(dst[:, csl], dst[:, csl], tmp[:], ADD)
                    for j in range(kpc):
                        kt = ci * kpc + j
                        vps = pv.tile([128, 128], f32, tag="v")
                        for a in range(n_din):
                            nc.tensor.matmul(
                                vps[:],
                                xts[:, a, j * 128 : (j + 1) * 128],
                                wv2s[:, a, :],
                                start=(a == 0),
                                stop=(a == n_din - 1),
                            )
                        nc.vector.tensor_copy(v2s[:, kt, 0:64], vps[:, 0:64])
                        nc.vector.tensor_copy(v2s[:, kt, 65:129], vps[:, 64:128])

            # ---- phase 2: causal attention (transposed-scores flash style) ----
            with (
                tc.tile_pool(name="att", bufs=4) as pa,
                tc.tile_pool(name="nrm", bufs=2) as pn,
                tc.tile_pool(name="sps", bufs=2, space="PSUM") as psp,
                tc.tile_pool(name="ops", bufs=2, space="PSUM") as pop,
            ):
                for h in (0, 1):
                    hp = slice(64 * h, 64 * h + 64)
                    vsl = slice(65 * h, 65 * h + 65)
                    for qt in range(n_ch):
                        qsl = slice(qt * CH, (qt + 1) * CH)
                        ops_ = pop.tile([65, CH], f32, tag="o")
                        base = qt * kpc
                        nk = base + kpc
                        for kp in range(nk // 2):
                            sps_ = psp.tile([128, 2, CH], f32, tag="s")
                            for j in (0, 1):
                                kt = 2 * kp + j
                                nc.tensor.matmul(
                                    sps_[:, j, :],
                                    kT2[hp, kt * 128 : (kt + 1) * 128],
                                    qT2[hp, qsl],
                                    start=True,
                                    stop=True,
                                )
                            pt = pa.tile([128, 2, CH], f32r, tag="pt")
                            nc.scalar.activation(pt[:], sps_[:], EXP, scale=0.125)
                            if 2 * kp >= base:
                                jj = 2 * kp - base
                                nc.vector.tensor_tensor(
                                    pt[:], pt[:], mask4s[:, jj : jj + 2, :], MUL
                                )
                            for j in (0, 1):
                                kt = 2 * kp + j
                                nc.tensor.matmul(
                                    ops_[:],
                                    v2s[:, kt, vsl],
                                    pt[:, j, :],
                                    start=(kt == 0),
                                    stop=(kt == nk - 1),
                                )
                        rec = pn.tile([1, CH], f32, tag="rec")
                        nc.vector.reciprocal(rec[:], ops_[64:65, :])
                        recb = pn.tile([64, CH], f32, tag="recb")
                        nc.gpsimd.partition_broadcast(recb[:], rec[:])
                        nc.vector.tensor_tensor(
                            yT2[hp, qsl], ops_[0:64, :], recb[:], MUL
                        )

            # ---- phase 3: output projection (row-parallel partial) ----
            with (
                tc.tile_pool(name="proj", bufs=4) as pj,
                tc.tile_pool(name="pps", bufs=4, space="PSUM") as ppp,
            ):
                for ti in range(n_kt):
                    for n_ in range(D // CH):
                        pps_ = ppp.tile([128, CH], f32, tag="p")
                        nc.tensor.matmul(
                            pps_[:],
                            yT2[:, ti * 128 : (ti + 1) * 128],
                            wp2s[:, n_ * CH : (n_ + 1) * CH],
                            start=True,
                            stop=True,
                        )
                        ev = pj.tile([128, CH], f32, tag="ev")
                        if n_ % 2 == 0:
                            nc.scalar.copy(ev[:], pps_[:])
                        else:
                            nc.vector.tensor_copy(ev[:], pps_[:])
                        nc.sync.dma_start(
                            out[ti * 128 : (ti + 1) * 128, n_ * CH : (n_ + 1) * CH],
                            ev[:],
                        )

    nc.compile()
    return nc


def host_inputs(x, w_qkv, w_proj, t=T):
    """Shard + lay out the full inputs into the 8 per-core input maps."""
    x2 = np.asarray(x, dtype=np.float32).reshape(t, D)
    w_qkv = np.asarray(w_qkv, dtype=np.float32)
    w_proj = np.asarray(w_proj, dtype=np.float32)
    xT = np.ascontiguousarray(x2.T)

    inv = 1.0 / (10000.0 ** (np.arange(0, HD, 2, dtype=np.float64) / HD))
    fr = np.outer(np.arange(t, dtype=np.float64), inv)  # [t, 32]
    cosT = np.concatenate([np.cos(fr), np.cos(fr)], axis=1).T  # [64, t]
    sinT = np.concatenate([np.sin(fr), np.sin(fr)], axis=1).T
    s2T = np.concatenate([-sinT[:32], sinT[32:]], axis=0)  # sign-folded
    cos2 = np.ascontiguousarray(np.concatenate([cosT, cosT], 0), dtype=np.float32)
    sin2 = np.ascontiguousarray(np.concatenate([s2T, s2T], 0), dtype=np.float32)

    rr = np.arange(128)[:, None]
    cc = np.arange(CH)[None, :]
    kpc = CH // KT
    mask = np.stack(
        [(128 * j + rr <= cc).astype(np.float32) for j in range(kpc)], axis=1
    )  # [128, kpc, CH]
    mask = np.ascontiguousarray(mask)

    wq = w_qkv[:, :D]
    wk = w_qkv[:, D : 2 * D]
    wv = w_qkv[:, 2 * D :]
    in_maps = []
    for c in range(N_CORES):
        sl = slice(128 * c, 128 * c + 128)
        in_maps.append(
            {
                "xT": xT,
                "wq2": np.ascontiguousarray(wq[:, sl]),
                "wk2": np.ascontiguousarray(wk[:, sl]),
                "wv2": np.ascontiguousarray(wv[:, sl]),
                "wp2": np.ascontiguousarray(w_proj[sl, :]),
                "cos2": cos2,
                "sin2": sin2,
                "mask": mask,
            }
        )
    return in_maps


def run(nc, in_maps, trace=False):
    from concourse.bass_utils import run_bass_kernel_spmd

    if trace:
        _ensure_ntff_hook()
    return run_bass_kernel_spmd(
        nc, in_maps, core_ids=list(range(N_CORES)), trace=trace
    )


def kernel(x, w_qkv, w_proj):
    if "nc" not in _CACHE:
        _CACHE["nc"] = build(T)
    nc = _CACHE["nc"]
    in_maps = host_inputs(x, w_qkv, w_proj, T)
    res = run(nc, in_maps)
    acc = res.results[0]["out"].astype(np.float32).copy()
    for c in range(1, N_CORES):
        acc += res.results[c]["out"]
    return acc.reshape(B, T, D)


# revision 6
# speedup vs baseline: 1.4229x; 1.4229x over previous
"""Causal self-attention (B=1, T=4096, D=1024, H=16, rotate-half RoPE)
as a head-parallel (tensor-parallel) Bass kernel on 8 TRN2 NeuronCores.

Sharding: core c computes heads {2c, 2c+1}:
  - w_qkv column-shard: each core gets its 128 q / 128 k / 128 v columns
  - w_proj row-shard: each core gets rows [128c, 128c+128)
  - each core emits a partial [T, D] output; host sums the 8 partials
    (the "all-reduce after proj" done on the host during unshard).

Per-core layout strategy (all fp32 storage, float32r matmuls):
  - x is passed transposed (xT [D, T]) so every matmul consumes it
    directly (contraction on partitions).
  - q, k are produced transposed+RoPEd: qT2/kT2 [128=2*64hd, T].
  - v is produced natural: v2s [128=T-tile, n_kt, 65*2] with a ones
    column appended per head (gives the softmax denominator for free).
  - scores are computed transposed: S^T [k, q] = kT.T @ qT, exp on ACT
    (scale=1/8 folded in), causal masking via multiplicative 0/1 masks
    on diagonal tiles only (off-causal tiles are never computed).
  - O^T [hd, q] = (v|1).T @ exp(S^T), accumulated over k tiles in PSUM;
    row 64 is the denominator; normalize with DVE reciprocal +
    gpsimd partition-broadcast.
  - proj: out[t, :] = yT2[:, t-tile].T @ w_proj_shard, evacuated
    PSUM->SBUF->HBM.
"""

import math

import numpy as np

B, T, D = 1, 4096, 1024
H = 16
HD = D // H  # 64
N_CORES = 8
CH = 512  # q-chunk (free dim of S^T matmuls)
KT = 128  # k-tile (contraction tile of the PV matmul)

MM_DT = "bf16"  # "f32r" | "bf16" — dtype of all matmul operands

_CACHE: dict = {}


def _ensure_ntff_hook():
    """Register antenv.axon_hooks if the image lacks it (needed only for
    trace=True profiling; harmless otherwise)."""
    import sys
    import types

    try:
        from antenv.axon_hooks import get_axon_ntff_profile_hook  # noqa: F401

        return
    except ImportError:
        pass
    mod = types.ModuleType("antenv.axon_hooks")
    _state = {"hook": None}
    mod.set_axon_ntff_profile_hook = lambda h: _state.__setitem__("hook", h)
    mod.get_axon_ntff_profile_hook = lambda: _state["hook"]
    sys.modules["antenv.axon_hooks"] = mod
    try:
        import antenv

        antenv.axon_hooks = mod
    except ImportError:
        pass
    try:
        from trn_agent_boot.trn_boot import _ntff_profile_via_ctypes

        hook = _ntff_profile_via_ctypes("/opt/axon/libaxon_pjrt.so")
        mod.set_axon_ntff_profile_hook(hook)
    except Exception:
        pass


def build(t=T):
    """Build + compile the per-core SPMD Bass program. Identical program on
    every core; only the weight shards in the input map differ."""
    import concourse.bacc as bacc
    import concourse.mybir as mybir
    import concourse.tile as tile

    f32 = mybir.dt.float32
    f32r = mybir.dt.float32r if MM_DT == "f32r" else mybir.dt.bfloat16
    MUL = mybir.AluOpType.mult
    ADD = mybir.AluOpType.add
    EXP = mybir.ActivationFunctionType.Exp

    n_ch = t // CH  # q-chunks
    n_kt = t // KT  # k-tiles / T-tiles
    n_din = D // 128  # contraction tiles over model dim
    kpc = CH // KT  # k-tiles per q-chunk (4)

    nc = bacc.Bacc(
        "TRN2", target_bir_lowering=False, debug=False, num_devices=N_CORES
    )
    xT = nc.dram_tensor("xT", [D, t], f32r, kind="ExternalInput").ap()
    wq2 = nc.dram_tensor("wq2", [D, 128], f32r, kind="ExternalInput").ap()
    wk2 = nc.dram_tensor("wk2", [D, 128], f32r, kind="ExternalInput").ap()
    wv2 = nc.dram_tensor("wv2", [D, 128], f32r, kind="ExternalInput").ap()
    wp2 = nc.dram_tensor("wp2", [128, D], f32r, kind="ExternalInput").ap()
    cos2 = nc.dram_tensor("cos2", [128, t], f32, kind="ExternalInput").ap()
    sin2 = nc.dram_tensor("sin2", [128, t], f32, kind="ExternalInput").ap()
    mask = nc.dram_tensor("mask", [128, kpc, CH], f32r, kind="ExternalInput").ap()
    out = nc.dram_tensor("out", [t, D], f32, kind="ExternalOutput").ap()

    with tile.TileContext(nc) as tc:
        with (
            tc.tile_pool(name="w", bufs=1) as pw,
            tc.tile_pool(name="big", bufs=1) as pb,
        ):
            wq2s = pw.tile([128, n_din, 128], f32r, tag="wq")
            nc.sync.dma_start(wq2s[:], wq2.rearrange("(a p) m -> p a m", p=128))
            wk2s = pw.tile([128, n_din, 128], f32r, tag="wk")
            nc.sync.dma_start(wk2s[:], wk2.rearrange("(a p) m -> p a m", p=128))
            wv2s = pw.tile([128, n_din, 128], f32r, tag="wv")
            nc.sync.dma_start(wv2s[:], wv2.rearrange("(a p) m -> p a m", p=128))
            wp2s = pw.tile([128, D], f32r, tag="wp")
            nc.sync.dma_start(wp2s[:], wp2)
            mask4s = pw.tile([128, kpc, CH], f32r, tag="mask")
            nc.sync.dma_start(mask4s[:], mask)

            qT2 = pb.tile([128, t], f32r, tag="qT2")
            kT2 = pb.tile([128, t], f32r, tag="kT2")
            v2s = pb.tile([128, n_kt, 130], f32r, tag="v2s")
            yT2 = pb.tile([128, t], f32r, tag="yT2")
            ones = pw.tile([128, n_kt, 1], f32, tag="ones")
            nc.vector.memset(ones[:], 1.0)
            nc.vector.tensor_copy(v2s[:, :, 64:65], ones[:])
            nc.vector.tensor_copy(v2s[:, :, 129:130], ones[:])

            # ---- phase 1: qkv projection + RoPE ----
            with (
                tc.tile_pool(name="rope", bufs=1) as pr,
                tc.tile_pool(name="x", bufs=2) as px,
                tc.tile_pool(name="rt", bufs=2) as prt,
                tc.tile_pool(name="qkps", bufs=3, space="PSUM") as pqk,
                tc.tile_pool(name="vps", bufs=2, space="PSUM") as pv,
            ):
                coss = pr.tile([128, t], f32, tag="cos")
                nc.sync.dma_start(coss[:], cos2)
                s2s = pr.tile([128, t], f32, tag="sin")
                nc.sync.dma_start(s2s[:], sin2)
                xTr = xT.rearrange("(a p) n -> p a n", p=128)
                for ci in range(n_ch):
                    csl = slice(ci * CH, (ci + 1) * CH)
                    xts = px.tile([128, n_din, CH], f32r, tag="x")
                    nc.sync.dma_start(xts[:], xTr[:, :, csl])
                    for dst, wts in ((qT2, wq2s), (kT2, wk2s)):
                        ps_ = pqk.tile([128, CH], f32, tag="qk")
                        for a in range(n_din):
                            nc.tensor.matmul(
                                ps_[:],
                                wts[:, a, :],
                                xts[:, a, :],
                                start=(a == 0),
                                stop=(a == n_din - 1),
                            )
                        # RoPE: dst = ps*cos + shift32(ps)*sin' (sin' sign-folded)
                        nc.vector.tensor_tensor(dst[:, csl], ps_[:], coss[:, csl], MUL)
                        tmp = prt.tile([128, CH], f32, tag="rt")
                        for d0, s0 in ((0, 32), (32, 0), (64, 96), (96, 64)):
                            nc.vector.tensor_tensor(
                                tmp[d0 : d0 + 32, :],
                                ps_[s0 : s0 + 32, :],
                                s2s[d0 : d0 + 32, csl],
                                MUL,
                            )
                        nc.vector.tensor_tensor(dst[:, csl], dst[:, csl], tmp[:], ADD)
                    for j in range(kpc):
                        kt = ci * kpc + j
                        vps = pv.tile([128, 128], f32, tag="v")
                        for a in range(n_din):
                            nc.tensor.matmul(
                                vps[:],
                                xts[:, a, j * 128 : (j + 1) * 128],
                                wv2s[:, a, :],
                                start=(a == 0),
                                stop=(a == n_din - 1),
                            )
                        nc.vector.tensor_copy(v2s[:, kt, 0:64], vps[:, 0:64])
                        nc.vector.tensor_copy(v2s[:, kt, 65:129], vps[:, 64:128])

            # ---- phase 2: causal attention (transposed-scores flash style) ----
            with (
                tc.tile_pool(name="att", bufs=4) as pa,
                tc.tile_pool(name="nrm", bufs=2) as pn,
                tc.tile_pool(name="sps", bufs=2, space="PSUM") as psp,
                tc.tile_pool(name="ops", bufs=2, space="PSUM") as pop,
            ):
                for h in (0, 1):
                    hp = slice(64 * h, 64 * h + 64)
                    vsl = slice(65 * h, 65 * h + 65)
                    for qt in range(n_ch):
                        qsl = slice(qt * CH, (qt + 1) * CH)
                        ops_ = pop.tile([65, CH], f32, tag="o")
                        base = qt * kpc
                        nk = base + kpc
                        for kp in range(nk // 2):
                            sps_ = psp.tile([128, 2, CH], f32, tag="s")
                            for j in (0, 1):
                                kt = 2 * kp + j
                                nc.tensor.matmul(
                                    sps_[:, j, :],
                                    kT2[hp, kt * 128 : (kt + 1) * 128],
                                    qT2[hp, qsl],
                                    start=True,
                                    stop=True,
                                )
                            pt = pa.tile([128, 2, CH], f32r, tag="pt")
                            nc.scalar.activation(pt[:], sps_[:], EXP, scale=0.125)
                            if 2 * kp >= base:
                                jj = 2 * kp - base
                                nc.vector.tensor_tensor(
                                    pt[:], pt[:], mask4s[:, jj : jj + 2, :], MUL
                                )
                            for j in (0, 1):
                                kt = 2 * kp + j
                                nc.tensor.matmul(
                                    ops_[:],
                                    v2s[:, kt, vsl],
                                    pt[:, j, :],
                                    start=(kt == 0),
                                    stop=(kt == nk - 1),
                                )
                        den = pn.tile([1, CH], f32, tag="den")
                        nc.vector.tensor_copy(den[:], ops_[64:65, :])
                        rec = pn.tile([1, CH], f32, tag="rec")
                        nc.vector.reciprocal_approx_fast(rec[:], den[:])
                        recb = pn.tile([64, CH], f32, tag="recb")
                        nc.gpsimd.partition_broadcast(recb[:], rec[:])
                        nc.vector.tensor_tensor(
                            yT2[hp, qsl], ops_[0:64, :], recb[:], MUL
                        )

            # ---- phase 3: output projection (row-parallel partial) ----
            with (
                tc.tile_pool(name="proj", bufs=4) as pj,
                tc.tile_pool(name="pps", bufs=4, space="PSUM") as ppp,
            ):
                for ti in range(n_kt):
                    for n_ in range(D // CH):
                        pps_ = ppp.tile([128, CH], f32, tag="p")
                        nc.tensor.matmul(
                            pps_[:],
                            yT2[:, ti * 128 : (ti + 1) * 128],
                            wp2s[:, n_ * CH : (n_ + 1) * CH],
                            start=True,
                            stop=True,
                        )
                        ev = pj.tile([128, CH], f32, tag="ev")
                        if n_ % 2 == 0:
                            nc.scalar.copy(ev[:], pps_[:])
                        else:
                            nc.vector.tensor_copy(ev[:], pps_[:])
                        nc.sync.dma_start(
                            out[ti * 128 : (ti + 1) * 128, n_ * CH : (n_ + 1) * CH],
                            ev[:],
                        )

    nc.compile()
    return nc


def host_inputs(x, w_qkv, w_proj, t=T):
    """Shard + lay out the full inputs into the 8 per-core input maps."""
    x2 = np.asarray(x, dtype=np.float32).reshape(t, D)
    w_qkv = np.asarray(w_qkv, dtype=np.float32)
    w_proj = np.asarray(w_proj, dtype=np.float32)
    xT = np.ascontiguousarray(x2.T)

    inv = 1.0 / (10000.0 ** (np.arange(0, HD, 2, dtype=np.float64) / HD))
    fr = np.outer(np.arange(t, dtype=np.float64), inv)  # [t, 32]
    cosT = np.concatenate([np.cos(fr), np.cos(fr)], axis=1).T  # [64, t]
    sinT = np.concatenate([np.sin(fr), np.sin(fr)], axis=1).T
    s2T = np.concatenate([-sinT[:32], sinT[32:]], axis=0)  # sign-folded
    cos2 = np.ascontiguousarray(np.concatenate([cosT, cosT], 0), dtype=np.float32)
    sin2 = np.ascontiguousarray(np.concatenate([s2T, s2T], 0), dtype=np.float32)

    rr = np.arange(128)[:, None]
    cc = np.arange(CH)[None, :]
    kpc = CH // KT
    mask = np.stack(
        [(128 * j + rr <= cc).astype(np.float32) for j in range(kpc)], axis=1
    )  # [128, kpc, CH]
    mask = np.ascontiguousarray(mask)

    if MM_DT == "bf16":
        import ml_dtypes

        bf = ml_dtypes.bfloat16
        xT = xT.astype(bf)
        mask = mask.astype(bf)
        cast = lambda a: np.ascontiguousarray(a).astype(bf)
    else:
        cast = np.ascontiguousarray
    wq = w_qkv[:, :D]
    wk = w_qkv[:, D : 2 * D]
    wv = w_qkv[:, 2 * D :]
    in_maps = []
    for c in range(N_CORES):
        sl = slice(128 * c, 128 * c + 128)
        in_maps.append(
            {
                "xT": xT,
                "wq2": cast(wq[:, sl]),
                "wk2": cast(wk[:, sl]),
                "wv2": cast(wv[:, sl]),
                "wp2": cast(w_proj[sl, :]),
                "cos2": cos2,
                "sin2": sin2,
                "mask": mask,
            }
        )
    return in_maps


def run(nc, in_maps, trace=False):
    from concourse.bass_utils import run_bass_kernel_spmd

    if trace:
        _ensure_ntff_hook()
    return run_bass_kernel_spmd(
        nc, in_maps, core_ids=list(range(N_CORES)), trace=trace
    )


def kernel(x, w_qkv, w_proj):
    if "nc" not in _CACHE:
        _CACHE["nc"] = build(T)
    nc = _CACHE["nc"]
    in_maps = host_inputs(x, w_qkv, w_proj, T)
    res = run(nc, in_maps)
    acc = res.results[0]["out"].astype(np.float32).copy()
    for c in range(1, N_CORES):
        acc += res.results[c]["out"]
    return acc.reshape(B, T, D)


# revision 10
# speedup vs baseline: 1.4855x; 1.0440x over previous
"""Causal self-attention (B=1, T=4096, D=1024, H=16, rotate-half RoPE)
as a head-parallel (tensor-parallel) Bass kernel on 8 TRN2 NeuronCores.

Sharding: core c computes heads {2c, 2c+1}:
  - w_qkv column-shard: each core gets its 128 q / 128 k / 128 v columns
  - w_proj row-shard: each core gets rows [128c, 128c+128)
  - each core emits a partial [T, D] output; host sums the 8 partials
    (the "all-reduce after proj" done on the host during unshard).

Per-core layout strategy (all fp32 storage, float32r matmuls):
  - x is passed transposed (xT [D, T]) so every matmul consumes it
    directly (contraction on partitions).
  - q, k are produced transposed+RoPEd: qT2/kT2 [128=2*64hd, T].
  - v is produced natural: v2s [128=T-tile, n_kt, 65*2] with a ones
    column appended per head (gives the softmax denominator for free).
  - scores are computed transposed: S^T [k, q] = kT.T @ qT, exp on ACT
    (scale=1/8 folded in), causal masking via multiplicative 0/1 masks
    on diagonal tiles only (off-causal tiles are never computed).
  - O^T [hd, q] = (v|1).T @ exp(S^T), accumulated over k tiles in PSUM;
    row 64 is the denominator; normalize with DVE reciprocal +
    gpsimd partition-broadcast.
  - proj: out[t, :] = yT2[:, t-tile].T @ w_proj_shard, evacuated
    PSUM->SBUF->HBM.
"""

import math

import numpy as np

B, T, D = 1, 4096, 1024
H = 16
HD = D // H  # 64
N_CORES = 8
CH = 512  # q-chunk (free dim of S^T matmuls)
KT = 128  # k-tile (contraction tile of the PV matmul)

MM_DT = "bf16"  # "f32r" | "bf16" — dtype of all matmul operands

_CACHE: dict = {}


def _ensure_ntff_hook():
    """Register antenv.axon_hooks if the image lacks it (needed only for
    trace=True profiling; harmless otherwise)."""
    import sys
    import types

    try:
        from antenv.axon_hooks import get_axon_ntff_profile_hook  # noqa: F401

        return
    except ImportError:
        pass
    mod = types.ModuleType("antenv.axon_hooks")
    _state = {"hook": None}
    mod.set_axon_ntff_profile_hook = lambda h: _state.__setitem__("hook", h)
    mod.get_axon_ntff_profile_hook = lambda: _state["hook"]
    sys.modules["antenv.axon_hooks"] = mod
    try:
        import antenv

        antenv.axon_hooks = mod
    except ImportError:
        pass
    try:
        from trn_agent_boot.trn_boot import _ntff_profile_via_ctypes

        hook = _ntff_profile_via_ctypes("/opt/axon/libaxon_pjrt.so")
        mod.set_axon_ntff_profile_hook(hook)
    except Exception:
        pass


def build(t=T):
    """Build + compile the per-core SPMD Bass program. Identical program on
    every core; only the weight shards in the input map differ."""
    import concourse.bacc as bacc
    import concourse.mybir as mybir
    import concourse.tile as tile

    f32 = mybir.dt.float32
    f32r = mybir.dt.float32r if MM_DT == "f32r" else mybir.dt.bfloat16
    MUL = mybir.AluOpType.mult
    ADD = mybir.AluOpType.add
    EXP = mybir.ActivationFunctionType.Exp

    n_ch = t // CH  # q-chunks
    n_kt = t // KT  # k-tiles / T-tiles
    n_din = D // 128  # contraction tiles over model dim
    kpc = CH // KT  # k-tiles per q-chunk (4)

    nc = bacc.Bacc(
        "TRN2", target_bir_lowering=False, debug=False, num_devices=N_CORES
    )
    xT = nc.dram_tensor("xT", [D, t], f32r, kind="ExternalInput").ap()
    wq2 = nc.dram_tensor("wq2", [D, 128], f32r, kind="ExternalInput").ap()
    wk2 = nc.dram_tensor("wk2", [D, 128], f32r, kind="ExternalInput").ap()
    wv2 = nc.dram_tensor("wv2", [D, 128], f32r, kind="ExternalInput").ap()
    wp2 = nc.dram_tensor("wp2", [128, D], f32r, kind="ExternalInput").ap()
    cos2 = nc.dram_tensor("cos2", [128, t], f32, kind="ExternalInput").ap()
    sin2 = nc.dram_tensor("sin2", [128, t], f32, kind="ExternalInput").ap()
    mask = nc.dram_tensor("mask", [128, kpc, 2, CH], f32r, kind="ExternalInput").ap()
    out = nc.dram_tensor("out", [t, D], f32, kind="ExternalOutput").ap()

    with tile.TileContext(nc) as tc:
        with (
            tc.tile_pool(name="w", bufs=1) as pw,
            tc.tile_pool(name="big", bufs=1) as pb,
            tc.tile_pool(name="x", bufs=2) as px,
            tc.tile_pool(name="rt", bufs=2) as prt,
            tc.tile_pool(name="att", bufs=4) as pa,
            tc.tile_pool(name="nrm", bufs=3) as pn,
            tc.tile_pool(name="proj", bufs=4) as pj,
            tc.tile_pool(name="qkps", bufs=2, space="PSUM") as pqk,
            tc.tile_pool(name="sps", bufs=2, space="PSUM") as psp,
            tc.tile_pool(name="ops", bufs=1, space="PSUM") as pop,
        ):
            wq2s = pw.tile([128, n_din, 128], f32r, tag="wq")
            nc.sync.dma_start(wq2s[:], wq2.rearrange("(a p) m -> p a m", p=128))
            wk2s = pw.tile([128, n_din, 128], f32r, tag="wk")
            nc.sync.dma_start(wk2s[:], wk2.rearrange("(a p) m -> p a m", p=128))
            wv2s = pw.tile([128, n_din, 128], f32r, tag="wv")
            nc.sync.dma_start(wv2s[:], wv2.rearrange("(a p) m -> p a m", p=128))
            wp2s = pw.tile([128, D], f32r, tag="wp")
            nc.sync.dma_start(wp2s[:], wp2)
            mask8s = pw.tile([128, kpc, 2, CH], f32r, tag="mask")
            nc.sync.dma_start(mask8s[:], mask)
            coss = pw.tile([128, t], f32, tag="cos")
            nc.sync.dma_start(coss[:], cos2)
            s2s = pw.tile([128, t], f32, tag="sin")
            nc.sync.dma_start(s2s[:], sin2)
            ones = pw.tile([128, 1], f32, tag="ones")
            nc.vector.memset(ones[:], 1.0)

            qT2c = [pb.tile([128, CH], f32r, tag=f"qT2_{i}", name=f"qT2_{i}") for i in range(n_ch)]
            kT2c = [pb.tile([128, CH], f32r, tag=f"kT2_{i}", name=f"kT2_{i}") for i in range(n_ch)]
            v2st = [pb.tile([128, 130], f32r, tag=f"v2s_{i}", name=f"v2s_{i}") for i in range(n_kt)]
            yT2c = [pb.tile([128, CH], f32r, tag=f"yT2_{i}", name=f"yT2_{i}") for i in range(n_ch)]

            xTr = xT.rearrange("(a p) n -> p a n", p=128)
            for ci in range(n_ch):
                csl = slice(ci * CH, (ci + 1) * CH)
                # ---- qkv projection + RoPE for chunk ci ----
                xts = px.tile([128, n_din, CH], f32r, tag="x")
                nc.sync.dma_start(xts[:], xTr[:, :, csl])
                for dst, wts in ((qT2c[ci], wq2s), (kT2c[ci], wk2s)):
                    ps_ = pqk.tile([128, CH], f32, tag="qk")
                    for a in range(n_din):
                        nc.tensor.matmul(
                            ps_[:],
                            wts[:, a, :],
                            xts[:, a, :],
                            start=(a == 0),
                            stop=(a == n_din - 1),
                        )
                    # RoPE: dst = ps*cos + shift32(ps)*sin' (sin' sign-folded)
                    nc.vector.tensor_tensor(dst[:], ps_[:], coss[:, csl], MUL)
                    tmp = prt.tile([128, CH], f32, tag="rt")
                    for d0, s0 in ((0, 32), (32, 0), (64, 96), (96, 64)):
                        nc.vector.tensor_tensor(
                            tmp[d0 : d0 + 32, :],
                            ps_[s0 : s0 + 32, :],
                            s2s[d0 : d0 + 32, csl],
                            MUL,
                        )
                    nc.vector.tensor_tensor(dst[:], dst[:], tmp[:], ADD)
                for j in range(kpc):
                    kt = ci * kpc + j
                    vps = pqk.tile([128, CH], f32, tag="qk")
                    for a in range(n_din):
                        nc.tensor.matmul(
                            vps[:, 0:128],
                            xts[:, a, j * 128 : (j + 1) * 128],
                            wv2s[:, a, :],
                            start=(a == 0),
                            stop=(a == n_din - 1),
                        )
                    nc.vector.tensor_copy(v2st[kt][:, 0:64], vps[:, 0:64])
                    nc.vector.tensor_copy(v2st[kt][:, 65:129], vps[:, 64:128])
                    nc.vector.tensor_copy(v2st[kt][:, 64:65], ones[:])
                    nc.vector.tensor_copy(v2st[kt][:, 129:130], ones[:])

                # ---- attention for q-chunk qt=ci, both heads (row-packed S^T) ----
                qt = ci
                base = qt * kpc
                nk = base + kpc
                oph = [
                    pop.tile([65, CH], f32, tag="o0", name=f"o0_{qt}"),
                    pop.tile([65, CH], f32, tag="o1", name=f"o1_{qt}"),
                ]
                for kt in range(nk):
                    kj = kt % kpc
                    kc = kt // kpc
                    sps_ = psp.tile([128, 2, CH], f32, tag="s")
                    for h in (0, 1):
                        hp = slice(64 * h, 64 * h + 64)
                        nc.tensor.matmul(
                            sps_[:, h, :],
                            kT2c[kc][hp, kj * 128 : (kj + 1) * 128],
                            qT2c[qt][hp, :],
                            start=True,
                            stop=True,
                        )
                    pt = pa.tile([128, 2, CH], f32r, tag="pt")
                    nc.scalar.activation(pt[:], sps_[:], EXP, scale=0.125)
                    if kt >= base:
                        nc.vector.tensor_tensor(
                            pt[:], pt[:], mask8s[:, kt - base, :, :], MUL
                        )
                    for h in (0, 1):
                        nc.tensor.matmul(
                            oph[h][:],
                            v2st[kt][:, 65 * h : 65 * h + 65],
                            pt[:, h, :],
                            start=(kt == 0),
                            stop=(kt == nk - 1),
                        )
                for h in (0, 1):
                    hp = slice(64 * h, 64 * h + 64)
                    den = pn.tile([1, CH], f32, tag="den")
                    nc.vector.tensor_copy(den[:], oph[h][64:65, :])
                    rec = pn.tile([1, CH], f32, tag="rec")
                    nc.vector.reciprocal_approx_fast(rec[:], den[:])
                    recb = pn.tile([64, CH], f32, tag="recb")
                    nc.gpsimd.partition_broadcast(recb[:], rec[:])
                    nc.vector.tensor_tensor(
                        yT2c[qt][hp, :], oph[h][0:64, :], recb[:], MUL
                    )

                # ---- output projection for chunk ci ----
                for tj in range(kpc):
                    ti = ci * kpc + tj
                    for n_ in range(D // CH):
                        pps_ = pqk.tile([128, CH], f32, tag="qk")
                        nc.tensor.matmul(
                            pps_[:],
                            yT2c[ci][:, tj * 128 : (tj + 1) * 128],
                            wp2s[:, n_ * CH : (n_ + 1) * CH],
                            start=True,
                            stop=True,
                        )
                        ev = pj.tile([128, CH], f32, tag="ev")
                        if n_ % 2 == 0:
                            nc.scalar.copy(ev[:], pps_[:])
                        else:
                            nc.vector.tensor_copy(ev[:], pps_[:])
                        nc.sync.dma_start(
                            out[ti * 128 : (ti + 1) * 128, n_ * CH : (n_ + 1) * CH],
                            ev[:],
                        )

    nc.compile()
    return nc


def host_inputs(x, w_qkv, w_proj, t=T):
    """Shard + lay out the full inputs into the 8 per-core input maps."""
    x2 = np.asarray(x, dtype=np.float32).reshape(t, D)
    w_qkv = np.asarray(w_qkv, dtype=np.float32)
    w_proj = np.asarray(w_proj, dtype=np.float32)
    xT = np.ascontiguousarray(x2.T)

    inv = 1.0 / (10000.0 ** (np.arange(0, HD, 2, dtype=np.float64) / HD))
    fr = np.outer(np.arange(t, dtype=np.float64), inv)  # [t, 32]
    cosT = np.concatenate([np.cos(fr), np.cos(fr)], axis=1).T  # [64, t]
    sinT = np.concatenate([np.sin(fr), np.sin(fr)], axis=1).T
    s2T = np.concatenate([-sinT[:32], sinT[32:]], axis=0)  # sign-folded
    cos2 = np.ascontiguousarray(np.concatenate([cosT, cosT], 0), dtype=np.float32)
    sin2 = np.ascontiguousarray(np.concatenate([s2T, s2T], 0), dtype=np.float32)

    rr = np.arange(128)[:, None]
    cc = np.arange(CH)[None, :]
    kpc = CH // KT
    mask = np.stack(
        [(128 * j + rr <= cc).astype(np.float32) for j in range(kpc)], axis=1
    )  # [128, kpc, CH]
    mask = np.ascontiguousarray(
        np.repeat(mask[:, :, None, :], 2, axis=2)
    )  # [128, kpc, 2, CH]

    if MM_DT == "bf16":
        import ml_dtypes

        bf = ml_dtypes.bfloat16
        xT = xT.astype(bf)
        mask = mask.astype(bf)
        cast = lambda a: np.ascontiguousarray(a).astype(bf)
    else:
        cast = np.ascontiguousarray
    wq = w_qkv[:, :D]
    wk = w_qkv[:, D : 2 * D]
    wv = w_qkv[:, 2 * D :]
    in_maps = []
    for c in range(N_CORES):
        sl = slice(128 * c, 128 * c + 128)
        in_maps.append(
            {
                "xT": xT,
                "wq2": cast(wq[:, sl]),
                "wk2": cast(wk[:, sl]),
                "wv2": cast(wv[:, sl]),
                "wp2": cast(w_proj[sl, :]),
                "cos2": cos2,
                "sin2": sin2,
                "mask": mask,
            }
        )
    return in_maps


def run(nc, in_maps, trace=False):
    from concourse.bass_utils import run_bass_kernel_spmd

    if trace:
        _ensure_ntff_hook()
    return run_bass_kernel_spmd(
        nc, in_maps, core_ids=list(range(N_CORES)), trace=trace
    )


def kernel(x, w_qkv, w_proj):
    if "nc" not in _CACHE:
        _CACHE["nc"] = build(T)
    nc = _CACHE["nc"]
    in_maps = host_inputs(x, w_qkv, w_proj, T)
    res = run(nc, in_maps)
    acc = res.results[0]["out"].astype(np.float32).copy()
    for c in range(1, N_CORES):
        acc += res.results[c]["out"]
    return acc.reshape(B, T, D)


# revision 11
# speedup vs baseline: 1.5020x; 1.0111x over previous
"""Causal self-attention (B=1, T=4096, D=1024, H=16, rotate-half RoPE)
as a head-parallel (tensor-parallel) Bass kernel on 8 TRN2 NeuronCores.

Sharding: core c computes heads {2c, 2c+1}:
  - w_qkv column-shard: each core gets its 128 q / 128 k / 128 v columns
  - w_proj row-shard: each core gets rows [128c, 128c+128)
  - each core emits a partial [T, D] output; host sums the 8 partials
    (the "all-reduce after proj" done on the host during unshard).

Per-core layout strategy (all fp32 storage, float32r matmuls):
  - x is passed transposed (xT [D, T]) so every matmul consumes it
    directly (contraction on partitions).
  - q, k are produced transposed+RoPEd: qT2/kT2 [128=2*64hd, T].
  - v is produced natural: v2s [128=T-tile, n_kt, 65*2] with a ones
    column appended per head (gives the softmax denominator for free).
  - scores are computed transposed: S^T [k, q] = kT.T @ qT, exp on ACT
    (scale=1/8 folded in), causal masking via multiplicative 0/1 masks
    on diagonal tiles only (off-causal tiles are never computed).
  - O^T [hd, q] = (v|1).T @ exp(S^T), accumulated over k tiles in PSUM;
    row 64 is the denominator; normalize with DVE reciprocal +
    gpsimd partition-broadcast.
  - proj: out[t, :] = yT2[:, t-tile].T @ w_proj_shard, evacuated
    PSUM->SBUF->HBM.
"""

import math

import numpy as np

B, T, D = 1, 4096, 1024
H = 16
HD = D // H  # 64
N_CORES = 8
CH = 512  # q-chunk (free dim of S^T matmuls)
KT = 128  # k-tile (contraction tile of the PV matmul)

MM_DT = "bf16"  # "f32r" | "bf16" — dtype of all matmul operands

_CACHE: dict = {}


def _ensure_ntff_hook():
    """Register antenv.axon_hooks if the image lacks it (needed only for
    trace=True profiling; harmless otherwise)."""
    import sys
    import types

    try:
        from antenv.axon_hooks import get_axon_ntff_profile_hook  # noqa: F401

        return
    except ImportError:
        pass
    mod = types.ModuleType("antenv.axon_hooks")
    _state = {"hook": None}
    mod.set_axon_ntff_profile_hook = lambda h: _state.__setitem__("hook", h)
    mod.get_axon_ntff_profile_hook = lambda: _state["hook"]
    sys.modules["antenv.axon_hooks"] = mod
    try:
        import antenv

        antenv.axon_hooks = mod
    except ImportError:
        pass
    try:
        from trn_agent_boot.trn_boot import _ntff_profile_via_ctypes

        hook = _ntff_profile_via_ctypes("/opt/axon/libaxon_pjrt.so")
        mod.set_axon_ntff_profile_hook(hook)
    except Exception:
        pass


def build(t=T):
    """Build + compile the per-core SPMD Bass program. Identical program on
    every core; only the weight shards in the input map differ."""
    import concourse.bacc as bacc
    import concourse.mybir as mybir
    import concourse.tile as tile

    f32 = mybir.dt.float32
    f32r = mybir.dt.float32r if MM_DT == "f32r" else mybir.dt.bfloat16
    MUL = mybir.AluOpType.mult
    ADD = mybir.AluOpType.add
    EXP = mybir.ActivationFunctionType.Exp

    n_ch = t // CH  # q-chunks
    n_kt = t // KT  # k-tiles / T-tiles
    n_din = D // 128  # contraction tiles over model dim
    kpc = CH // KT  # k-tiles per q-chunk (4)

    nc = bacc.Bacc(
        "TRN2", target_bir_lowering=False, debug=False, num_devices=N_CORES
    )
    xT = nc.dram_tensor("xT", [D, t], f32r, kind="ExternalInput").ap()
    wq2 = nc.dram_tensor("wq2", [D, 128], f32r, kind="ExternalInput").ap()
    wk2 = nc.dram_tensor("wk2", [D, 128], f32r, kind="ExternalInput").ap()
    wv2 = nc.dram_tensor("wv2", [D, 128], f32r, kind="ExternalInput").ap()
    wp2 = nc.dram_tensor("wp2", [128, D], f32r, kind="ExternalInput").ap()
    cos2 = nc.dram_tensor("cos2", [128, t], f32, kind="ExternalInput").ap()
    sin2 = nc.dram_tensor("sin2", [128, t], f32, kind="ExternalInput").ap()
    mask = nc.dram_tensor("mask", [128, kpc, 2, CH], f32r, kind="ExternalInput").ap()
    out = nc.dram_tensor("out", [t, D], f32, kind="ExternalOutput").ap()

    with tile.TileContext(nc) as tc:
        with (
            tc.tile_pool(name="w", bufs=1) as pw,
            tc.tile_pool(name="big", bufs=1) as pb,
            tc.tile_pool(name="x", bufs=2) as px,
            tc.tile_pool(name="rt", bufs=2) as prt,
            tc.tile_pool(name="att", bufs=4) as pa,
            tc.tile_pool(name="nrm", bufs=3) as pn,
            tc.tile_pool(name="proj", bufs=4) as pj,
            tc.tile_pool(name="qkps", bufs=2, space="PSUM") as pqk,
            tc.tile_pool(name="sps", bufs=2, space="PSUM") as psp,
            tc.tile_pool(name="ops", bufs=1, space="PSUM") as pop,
        ):
            wq2s = pw.tile([128, n_din, 128], f32r, tag="wq")
            nc.sync.dma_start(wq2s[:], wq2.rearrange("(a p) m -> p a m", p=128))
            wk2s = pw.tile([128, n_din, 128], f32r, tag="wk")
            nc.sync.dma_start(wk2s[:], wk2.rearrange("(a p) m -> p a m", p=128))
            wv2s = pw.tile([128, n_din, 128], f32r, tag="wv")
            nc.sync.dma_start(wv2s[:], wv2.rearrange("(a p) m -> p a m", p=128))
            wp2s = pw.tile([128, D], f32r, tag="wp")
            nc.sync.dma_start(wp2s[:], wp2)
            mask8s = pw.tile([128, kpc, 2, CH], f32r, tag="mask")
            nc.sync.dma_start(mask8s[:], mask)
            coss = pw.tile([128, t], f32, tag="cos")
            nc.sync.dma_start(coss[:], cos2)
            s2s = pw.tile([128, t], f32, tag="sin")
            nc.sync.dma_start(s2s[:], sin2)
            ones = pw.tile([128, 1], f32, tag="ones")
            nc.vector.memset(ones[:], 1.0)

            qT2c = [pb.tile([128, CH], f32r, tag=f"qT2_{i}", name=f"qT2_{i}") for i in range(n_ch)]
            kT2c = [pb.tile([128, CH], f32r, tag=f"kT2_{i}", name=f"kT2_{i}") for i in range(n_ch)]
            v2st = [pb.tile([128, 130], f32r, tag=f"v2s_{i}", name=f"v2s_{i}") for i in range(n_kt)]
            yT2c = [pb.tile([128, CH], f32r, tag=f"yT2_{i}", name=f"yT2_{i}") for i in range(n_ch)]

            xTr = xT.rearrange("(a p) n -> p a n", p=128)

            def qkv_chunk(ci):
                csl = slice(ci * CH, (ci + 1) * CH)
                xts = px.tile([128, n_din, CH], f32r, tag="x", name=f"xts_{ci}")
                nc.sync.dma_start(xts[:], xTr[:, :, csl])
                for dst, wts in ((qT2c[ci], wq2s), (kT2c[ci], wk2s)):
                    ps_ = pqk.tile([128, CH], f32, tag="qk")
                    for a in range(n_din):
                        nc.tensor.matmul(
                            ps_[:],
                            wts[:, a, :],
                            xts[:, a, :],
                            start=(a == 0),
                            stop=(a == n_din - 1),
                        )
                    # RoPE: dst = ps*cos + shift32(ps)*sin' (sin' sign-folded)
                    nc.vector.tensor_tensor(dst[:], ps_[:], coss[:, csl], MUL)
                    tmp = prt.tile([128, CH], f32, tag="rt")
                    for d0, s0 in ((0, 32), (32, 0), (64, 96), (96, 64)):
                        nc.vector.tensor_tensor(
                            tmp[d0 : d0 + 32, :],
                            ps_[s0 : s0 + 32, :],
                            s2s[d0 : d0 + 32, csl],
                            MUL,
                        )
                    nc.vector.tensor_tensor(dst[:], dst[:], tmp[:], ADD)
                for j in range(kpc):
                    kt = ci * kpc + j
                    vps = pqk.tile([128, CH], f32, tag="qk")
                    for a in range(n_din):
                        nc.tensor.matmul(
                            vps[:, 0:128],
                            xts[:, a, j * 128 : (j + 1) * 128],
                            wv2s[:, a, :],
                            start=(a == 0),
                            stop=(a == n_din - 1),
                        )
                    nc.vector.tensor_copy(v2st[kt][:, 0:64], vps[:, 0:64])
                    nc.vector.tensor_copy(v2st[kt][:, 65:129], vps[:, 64:128])
                    nc.vector.tensor_copy(v2st[kt][:, 64:65], ones[:])
                    nc.vector.tensor_copy(v2st[kt][:, 129:130], ones[:])

            qkv_chunk(0)
            for ci in range(n_ch):
                # ---- attention for q-chunk qt=ci, both heads (row-packed S^T) ----
                qt = ci
                base = qt * kpc
                nk = base + kpc
                oph = [
                    pop.tile([65, CH], f32, tag="o0", name=f"o0_{qt}"),
                    pop.tile([65, CH], f32, tag="o1", name=f"o1_{qt}"),
                ]
                for kt in range(nk):
                    kj = kt % kpc
                    kc = kt // kpc
                    sps_ = psp.tile([128, 2, CH], f32, tag="s")
                    for h in (0, 1):
                        hp = slice(64 * h, 64 * h + 64)
                        nc.tensor.matmul(
                            sps_[:, h, :],
                            kT2c[kc][hp, kj * 128 : (kj + 1) * 128],
                            qT2c[qt][hp, :],
                            start=True,
                            stop=True,
                        )
                    pt = pa.tile([128, 2, CH], f32r, tag="pt")
                    nc.scalar.activation(pt[:], sps_[:], EXP, scale=0.125)
                    if kt >= base:
                        nc.vector.tensor_tensor(
                            pt[:], pt[:], mask8s[:, kt - base, :, :], MUL
                        )
                    for h in (0, 1):
                        nc.tensor.matmul(
                            oph[h][:],
                            v2st[kt][:, 65 * h : 65 * h + 65],
                            pt[:, h, :],
                            start=(kt == 0),
                            stop=(kt == nk - 1),
                        )
                for h in (0, 1):
                    hp = slice(64 * h, 64 * h + 64)
                    den = pn.tile([1, CH], f32, tag="den")
                    nc.vector.tensor_copy(den[:], oph[h][64:65, :])
                    rec = pn.tile([1, CH], f32, tag="rec")
                    nc.vector.reciprocal_approx_fast(rec[:], den[:])
                    recb = pn.tile([64, CH], f32, tag="recb")
                    nc.gpsimd.partition_broadcast(recb[:], rec[:])
                    nc.vector.tensor_tensor(
                        yT2c[qt][hp, :], oph[h][0:64, :], recb[:], MUL
                    )

                # ---- output projection for chunk ci ----
                for tj in range(kpc):
                    ti = ci * kpc + tj
                    for n_ in range(D // CH):
                        pps_ = pqk.tile([128, CH], f32, tag="qk")
                        nc.tensor.matmul(
                            pps_[:],
                            yT2c[ci][:, tj * 128 : (tj + 1) * 128],
                            wp2s[:, n_ * CH : (n_ + 1) * CH],
                            start=True,
                            stop=True,
                        )
                        ev = pj.tile([128, CH], f32, tag="ev")
                        nc.vector.tensor_copy(ev[:], pps_[:])
                        nc.sync.dma_start(
                            out[ti * 128 : (ti + 1) * 128, n_ * CH : (n_ + 1) * CH],
                            ev[:],
                        )
                if ci + 1 < n_ch:
                    qkv_chunk(ci + 1)

    nc.compile()
    return nc


def host_inputs(x, w_qkv, w_proj, t=T):
    """Shard + lay out the full inputs into the 8 per-core input maps."""
    x2 = np.asarray(x, dtype=np.float32).reshape(t, D)
    w_qkv = np.asarray(w_qkv, dtype=np.float32)
    w_proj = np.asarray(w_proj, dtype=np.float32)
    xT = np.ascontiguousarray(x2.T)

    inv = 1.0 / (10000.0 ** (np.arange(0, HD, 2, dtype=np.float64) / HD))
    fr = np.outer(np.arange(t, dtype=np.float64), inv)  # [t, 32]
    cosT = np.concatenate([np.cos(fr), np.cos(fr)], axis=1).T  # [64, t]
    sinT = np.concatenate([np.sin(fr), np.sin(fr)], axis=1).T
    s2T = np.concatenate([-sinT[:32], sinT[32:]], axis=0)  # sign-folded
    cos2 = np.ascontiguousarray(np.concatenate([cosT, cosT], 0), dtype=np.float32)
    sin2 = np.ascontiguousarray(np.concatenate([s2T, s2T], 0), dtype=np.float32)

    rr = np.arange(128)[:, None]
    cc = np.arange(CH)[None, :]
    kpc = CH // KT
    mask = np.stack(
        [(128 * j + rr <= cc).astype(np.float32) for j in range(kpc)], axis=1
    )  # [128, kpc, CH]
    mask = np.ascontiguousarray(
        np.repeat(mask[:, :, None, :], 2, axis=2)
    )  # [128, kpc, 2, CH]

    if MM_DT == "bf16":
        import ml_dtypes

        bf = ml_dtypes.bfloat16
        xT = xT.astype(bf)
        mask = mask.astype(bf)
        cast = lambda a: np.ascontiguousarray(a).astype(bf)
    else:
        cast = np.ascontiguousarray
    wq = w_qkv[:, :D]
    wk = w_qkv[:, D : 2 * D]
    wv = w_qkv[:, 2 * D :]
    in_maps = []
    for c in range(N_CORES):
        sl = slice(128 * c, 128 * c + 128)
        in_maps.append(
            {
                "xT": xT,
                "wq2": cast(wq[:, sl]),
                "wk2": cast(wk[:, sl]),
                "wv2": cast(wv[:, sl]),
                "wp2": cast(w_proj[sl, :]),
                "cos2": cos2,
                "sin2": sin2,
                "mask": mask,
            }
        )
    return in_maps


def run(nc, in_maps, trace=False):
    from concourse.bass_utils import run_bass_kernel_spmd

    if trace:
        _ensure_ntff_hook()
    return run_bass_kernel_spmd(
        nc, in_maps, core_ids=list(range(N_CORES)), trace=trace
    )


def kernel(x, w_qkv, w_proj):
    if "nc" not in _CACHE:
        _CACHE["nc"] = build(T)
    nc = _CACHE["nc"]
    in_maps = host_inputs(x, w_qkv, w_proj, T)
    res = run(nc, in_maps)
    acc = res.results[0]["out"].astype(np.float32).copy()
    for c in range(1, N_CORES):
        acc += res.results[c]["out"]
    return acc.reshape(B, T, D)
